# revision 44
# baseline (speedup 1.0000x reference)
"""DifferentialAttention Trainium2 kernel (8 NeuronCores, SPMD).

Sharding: data-parallel over batch B=4, tensor-parallel over heads
(2 cores per batch element, 8 heads each).  Each core computes the
partial projection output for its 8 heads; the host sums the two
partials per batch element and adds b_proj.

Per-core device pipeline (all matmuls bf16 inputs, fp32 PSUM accum):
  1. QKV^T = W_slice^T.T @ x^T            -> [channels, n] layout
  2. V transpose via PE (keys on partitions), ones column appended
  3. scores S^T[m, n] per (head, half) with 4-way row-group packing
     (contraction d=32 -> PE row groups 0/32/64/96)
  4. exp on ScalarE (scale=1/8 folded in), bf16 out
  5. PV:  [V | 1]^T @ E  -> unnormalized out^T + softmax denominator row
  6. combine: O^T = O1/d1 - lam*O2/d2 (reciprocal + GPSIMD partition
     broadcast + DVE mul/add)
  7. proj: out = O^T.T @ Wp_slice
"""

import sys

sys.path.insert(0, "/opt/trn_rl_repo")

import numpy as np
import ml_dtypes

B, N, C, H, HD = 4, 1024, 1024, 16, 64
LAMBDA_INIT = 0.8
BF16 = ml_dtypes.bfloat16

_PROG_CACHE = {}


def _build_program(loop_n=1, dma_outside=False, skip=()):
    key = ("nc", loop_n, dma_outside, tuple(skip))
    if key in _PROG_CACHE:
        return _PROG_CACHE[key]

    import contextlib

    import concourse.mybir as mybir
    import concourse.tile as tile
    from concourse import bacc

    f32 = mybir.dt.float32
    b16 = mybir.dt.bfloat16
    Exp = mybir.ActivationFunctionType.Exp

    nc = bacc.Bacc(None)

    x_d = nc.dram_tensor("xT", [8, 128, N], b16, kind="ExternalInput")
    wqkv_d = nc.dram_tensor("wqkv", [8, 128, 1536], b16, kind="ExternalInput")
    wp_d = nc.dram_tensor("wp", [4, 128, C], b16, kind="ExternalInput")
    neglam_d = nc.dram_tensor("neglam", [1, 1], f32, kind="ExternalInput")
    out_d = nc.dram_tensor("out", [8, 128, C], f32, kind="ExternalOutput")

    with tile.TileContext(nc) as tc:
        with (
            tc.tile_pool(name="io", bufs=1) as iopool,
            tc.tile_pool(name="work", bufs=3) as wpool,
            tc.tile_pool(name="esb", bufs=4) as epool,
            tc.tile_pool(name="pS", bufs=2, space="PSUM") as pS,
            tc.tile_pool(name="pV", bufs=1, space="PSUM") as pV,
        ):
            xT = iopool.tile([128, 8, N], b16)
            wqkv = iopool.tile([128, 8, 1536], b16)
            wp = iopool.tile([128, 4, C], b16)
            neglam = iopool.tile([1, 1], f32)
            ones1 = iopool.tile([1, 64], b16)
            # qkvT chunk j: j in 0..3 -> q head-pair j; 4..7 -> k head-pair
            # within a chunk: partitions 0-63 even head (d 0..63), 64-127 odd
            qkvT = iopool.tile([128, 8, N], b16)
            # V in [keys, channels] layout: [m%128, m//128, head_local, d|1]
            # col 64 of each head's block is the ones column (softmax denom)
            vsb = iopool.tile([128, 8, 8, 65], b16)
            # combined attention output (transposed): [cl, hp, n]
            oT = iopool.tile([128, 4, N], b16)

            nc.gpsimd.memset(ones1[:], 1.0)
            nc.gpsimd.memset(vsb[:, :, :, 64:65], 1.0)
            if "attn" in skip:
                nc.gpsimd.memset(oT[:], 0.0)

            def dma_inputs():
                for cc in range(8):
                    nc.sync.dma_start(xT[:, cc, :], x_d[cc])
                    nc.sync.dma_start(wqkv[:, cc, :], wqkv_d[cc])
                for ci in range(4):
                    nc.sync.dma_start(wp[:, ci, :], wp_d[ci])
                nc.sync.dma_start(neglam[:], neglam_d[:])

            def qkv_psum():
                # phases A/C borrow score-pool slots ([128,2,512] granularity)
                t = pS.tile([128, 2, 512], f32, tag="s", name="qkvps")
                return t[:, 0, :]

            if dma_outside:
                dma_inputs()
            loop_ctx = (
                tc.For_i(0, loop_n, 1) if loop_n > 1 else contextlib.nullcontext()
            )
            loop_ctx.__enter__()
            if not dma_outside:
                dma_inputs()

            # ---------------- Phase A: QKV projection -----------------
            if True:
                for hp in range(4) if "qkv" not in skip else []:
                    for t in range(2):  # q, k -> [channels, n] layout
                        j = t * 4 + hp
                        for nh in range(2):
                            ps = qkv_psum()
                            for cc in range(8):
                                nc.tensor.matmul(
                                    ps[:],
                                    wqkv[:, cc, j * 128 : (j + 1) * 128],
                                    xT[:, cc, nh * 512 : (nh + 1) * 512],
                                    start=(cc == 0),
                                    stop=(cc == 7),
                                )
                            nc.scalar.copy(
                                qkvT[:, j, nh * 512 : (nh + 1) * 512], ps[:]
                            )
                # v -> [keys, channels] layout (operands swapped)
                for mc in range(8) if "qkv" not in skip else []:
                    ps = qkv_psum()
                    for cc in range(8):
                        nc.tensor.matmul(
                            ps[:],
                            xT[:, cc, mc * 128 : (mc + 1) * 128],
                            wqkv[:, cc, 1024:1536],
                            start=(cc == 0),
                            stop=(cc == 7),
                        )
                    nc.scalar.copy(
                        vsb[:, mc, :, 0:64], ps.rearrange("p (g d) -> p g d", g=8)
                    )

            # ---------------- Phase B: attention ----------------------
            # combo order ci: 0=(even,h1) 1=(odd,h1) 2=(even,h2) 3=(odd,h2)
            # (rg, parity): rg = score row-group, parity selects V head
            combo = [(0, 0), (2, 1), (1, 0), (3, 1)]
            if True:
                for hp in range(4) if "attn" not in skip else []:
                    for nh in range(2):
                        pv = pV.tile([65, 4, 512], f32, tag="pv")

                        def emit_pv(mc, etiles):
                            for g in range(2):
                                for i in range(2):
                                    ci = 2 * g + i
                                    _rg, par = combo[ci]
                                    nc.tensor.matmul(
                                        pv[:, ci, :],
                                        vsb[:, mc, 2 * hp + par, :],
                                        etiles[g][:, i, :],
                                        start=(mc == 0),
                                        stop=(mc == 7),
                                    )

                        # software pipeline: PV for chunk mc is emitted after
                        # the scores/exp of chunk mc+1, so the PE never stalls
                        # on the current chunk's exp
                        prev = None
                        for mc in range(8):
                            cur = []
                            for g in range(2):
                                s_ps = pS.tile([128, 2, 512], f32, tag="s")
                                for i in range(2):
                                    rg, _par = combo[2 * g + i]
                                    nc.tensor.matmul(
                                        s_ps[:, i, :],
                                        qkvT[
                                            32 * rg : 32 * rg + 32,
                                            4 + hp,
                                            mc * 128 : (mc + 1) * 128,
                                        ],
                                        qkvT[
                                            32 * rg : 32 * rg + 32,
                                            hp,
                                            nh * 512 : (nh + 1) * 512,
                                        ],
                                        start=True,
                                        stop=True,
                                        tile_position=(32 * rg, 0),
                                    )
                                e_sb = epool.tile([128, 2, 512], b16, tag="e")
                                nc.scalar.activation(
                                    e_sb[:], s_ps[:], Exp, scale=0.125
                                )
                                cur.append(e_sb)
                            if prev is not None:
                                emit_pv(mc - 1, prev)
                            prev = cur
                        emit_pv(7, prev)
                        # evacuate pv to SBUF immediately so the next sweep's
                        # PV accumulation can reuse the PSUM banks; the whole
                        # combine below then runs off the critical path
                        pvs = wpool.tile([65, 4, 512], f32, tag="pvs")
                        nc.vector.tensor_copy(pvs[:], pv[:])
                        # combine: O = O1/d1 - lam*O2/d2
                        rsb = wpool.tile([1, 4, 512], b16, tag="rsb")
                        with nc.allow_low_precision(
                            reason="bf16 softmax denominator reciprocals"
                        ):
                            nc.vector.reciprocal(rsb[:], pvs[64:65, :, :])
                        nc.vector.tensor_scalar_mul(
                            rsb[0:1, 2:4, :], rsb[0:1, 2:4, :], neglam[:]
                        )
                        # broadcast recips across partitions via PE ones-matmul
                        # (reuses freed score-pool slots), then stage in SBUF
                        rbc = wpool.tile([64, 4, 512], f32, tag="rbc")
                        for g in range(2):
                            rbc_ps = pS.tile([128, 2, 512], f32, tag="s")
                            for i in range(2):
                                nc.tensor.matmul(
                                    rbc_ps[0:64, i, :],
                                    ones1[:],
                                    rsb[0:1, 2 * g + i, :],
                                    start=True,
                                    stop=True,
                                )
                            nc.vector.tensor_copy(
                                rbc[:, 2 * g : 2 * g + 2, :], rbc_ps[0:64, :, :]
                            )
                        for par in range(2):
                            c1, c2 = par, 2 + par
                            t0 = wpool.tile([64, 512], f32, tag="t0")
                            t1 = wpool.tile([64, 512], f32, tag="t1")
                            nc.vector.tensor_mul(
                                out=t0[:], in0=pvs[0:64, c1, :], in1=rbc[:, c1, :]
                            )
                            nc.vector.tensor_mul(
                                out=t1[:], in0=pvs[0:64, c2, :], in1=rbc[:, c2, :]
                            )
                            nc.vector.tensor_add(
                                out=oT[
                                    par * 64 : (par + 1) * 64,
                                    hp,
                                    nh * 512 : (nh + 1) * 512,
                                ],
                                in0=t0[:],
                                in1=t1[:],
                            )

            # ---------------- Phase C: output projection --------------
            if True:
                for ncc in range(8) if "proj" not in skip else []:
                    for jh in range(2):
                        ps = qkv_psum()
                        for ci in range(4):
                            nc.tensor.matmul(
                                ps[:],
                                oT[:, ci, ncc * 128 : (ncc + 1) * 128],
                                wp[:, ci, jh * 512 : (jh + 1) * 512],
                                start=(ci == 0),
                                stop=(ci == 3),
                            )
                        osb = wpool.tile([128, 512], f32, tag="osb")
                        nc.scalar.copy(osb[:], ps[:])
                        nc.sync.dma_start(
                            out_d[ncc, :, jh * 512 : (jh + 1) * 512], osb[:]
                        )

            loop_ctx.__exit__(None, None, None)

    nc.compile()
    _PROG_CACHE[key] = nc
    return nc


def _prep_core_inputs(x, W_qkv, W_proj, neg_lam):
    """Host-side shard prep. Returns in_maps for the 8 cores."""
    W4 = np.asarray(W_qkv, np.float32).reshape(3, H, HD, C)
    in_maps = []
    for core in range(8):
        b, hg = divmod(core, 2)
        xT = (
            np.ascontiguousarray(np.asarray(x[b], np.float32).T)
            .reshape(8, 128, N)
            .astype(BF16)
        )
        wsl = W4[:, hg * 8 : (hg + 1) * 8]  # [3, 8, 64, 1024]
        wqkv = (
            np.ascontiguousarray(wsl.transpose(3, 0, 1, 2).reshape(C, 1536))
            .reshape(8, 128, 1536)
            .astype(BF16)
        )
        wp = (
            np.ascontiguousarray(
                np.asarray(W_proj, np.float32)[:, hg * 512 : (hg + 1) * 512].T
            )
            .reshape(4, 128, C)
            .astype(BF16)
        )
        in_maps.append(
            {
                "xT": xT,
                "wqkv": wqkv,
                "wp": wp,
                "neglam": np.full((1, 1), neg_lam, np.float32),
            }
        )
    return in_maps


def kernel(x, W_qkv, W_proj, b_proj, lambda_q1, lambda_k1, lambda_q2, lambda_k2):
    from concourse.bass_utils import run_bass_kernel_spmd

    lq1 = np.asarray(lambda_q1, np.float64)
    lk1 = np.asarray(lambda_k1, np.float64)
    lq2 = np.asarray(lambda_q2, np.float64)
    lk2 = np.asarray(lambda_k2, np.float64)
    lam = float(np.mean(np.exp(lq1 * lk1) - np.exp(lq2 * lk2) + LAMBDA_INIT))

    nc = _build_program()
    in_maps = _prep_core_inputs(x, W_qkv, W_proj, -lam)
    res = run_bass_kernel_spmd(nc, in_maps, core_ids=list(range(8)))
    _PROG_CACHE["last_result"] = res

    bp = np.asarray(b_proj, np.float32)
    out = np.empty((B, N, C), np.float32)
    for b in range(B):
        p0 = res.results[2 * b]["out"].reshape(N, C)
        p1 = res.results[2 * b + 1]["out"].reshape(N, C)
        out[b] = p0 + p1 + bp[None, :]
    return out


# revision 47
# speedup vs baseline: 1.0007x; 1.0007x over previous
"""DifferentialAttention Trainium2 kernel (8 NeuronCores, SPMD).

Sharding: data-parallel over batch B=4, tensor-parallel over heads
(2 cores per batch element, 8 heads each).  Each core computes the
partial projection output for its 8 heads; the host sums the two
partials per batch element and adds b_proj.

Per-core device pipeline (all matmuls bf16 inputs, fp32 PSUM accum):
  1. QKV^T = W_slice^T.T @ x^T            -> [channels, n] layout
  2. V transpose via PE (keys on partitions), ones column appended
  3. scores S^T[m, n] per (head, half) with 4-way row-group packing
     (contraction d=32 -> PE row groups 0/32/64/96)
  4. exp on ScalarE (scale=1/8 folded in), bf16 out
  5. PV:  [V | 1]^T @ E  -> unnormalized out^T + softmax denominator row
  6. combine: O^T = O1/d1 - lam*O2/d2 (reciprocal + GPSIMD partition
     broadcast + DVE mul/add)
  7. proj: out = O^T.T @ Wp_slice
"""

import sys

sys.path.insert(0, "/opt/trn_rl_repo")

import numpy as np
import ml_dtypes

B, N, C, H, HD = 4, 1024, 1024, 16, 64
LAMBDA_INIT = 0.8
BF16 = ml_dtypes.bfloat16

_PROG_CACHE = {}


def _build_program(loop_n=1, dma_outside=False, skip=()):
    key = ("nc", loop_n, dma_outside, tuple(skip))
    if key in _PROG_CACHE:
        return _PROG_CACHE[key]

    import contextlib

    import concourse.mybir as mybir
    import concourse.tile as tile
    from concourse import bacc

    f32 = mybir.dt.float32
    b16 = mybir.dt.bfloat16
    Exp = mybir.ActivationFunctionType.Exp

    nc = bacc.Bacc(None)

    x_d = nc.dram_tensor("xT", [8, 128, N], b16, kind="ExternalInput")
    wqkv_d = nc.dram_tensor("wqkv", [8, 128, 1536], b16, kind="ExternalInput")
    wp_d = nc.dram_tensor("wp", [4, 128, C], b16, kind="ExternalInput")
    neglam_d = nc.dram_tensor("neglam", [1, 1], f32, kind="ExternalInput")
    out_d = nc.dram_tensor("out", [8, 128, C], f32, kind="ExternalOutput")

    with tile.TileContext(nc) as tc:
        with (
            tc.tile_pool(name="io", bufs=1) as iopool,
            tc.tile_pool(name="work", bufs=4) as wpool,
            tc.tile_pool(name="esb", bufs=6) as epool,
            tc.tile_pool(name="pS", bufs=2, space="PSUM") as pS,
            tc.tile_pool(name="pV", bufs=1, space="PSUM") as pV,
        ):
            xT = iopool.tile([128, 8, N], b16)
            wqkv = iopool.tile([128, 8, 1536], b16)
            wp = iopool.tile([128, 4, C], b16)
            neglam = iopool.tile([1, 1], f32)
            ones1 = iopool.tile([1, 64], b16)
            # qkvT chunk j: j in 0..3 -> q head-pair j; 4..7 -> k head-pair
            # within a chunk: partitions 0-63 even head (d 0..63), 64-127 odd
            qkvT = iopool.tile([128, 8, N], b16)
            # V in [keys, channels] layout: [m%128, m//128, head_local, d|1]
            # col 64 of each head's block is the ones column (softmax denom)
            vsb = iopool.tile([128, 8, 8, 65], b16)
            # combined attention output (transposed): [cl, hp, n]
            oT = iopool.tile([128, 4, N], b16)

            nc.gpsimd.memset(ones1[:], 1.0)
            nc.gpsimd.memset(vsb[:, :, :, 64:65], 1.0)
            if "attn" in skip:
                nc.gpsimd.memset(oT[:], 0.0)

            def dma_inputs():
                for cc in range(8):
                    nc.sync.dma_start(xT[:, cc, :], x_d[cc])
                    nc.sync.dma_start(wqkv[:, cc, :], wqkv_d[cc])
                for ci in range(4):
                    nc.sync.dma_start(wp[:, ci, :], wp_d[ci])
                nc.sync.dma_start(neglam[:], neglam_d[:])

            def qkv_psum():
                # phases A/C borrow score-pool slots ([128,2,512] granularity)
                t = pS.tile([128, 2, 512], f32, tag="s", name="qkvps")
                return t[:, 0, :]

            if dma_outside:
                dma_inputs()
            loop_ctx = (
                tc.For_i(0, loop_n, 1) if loop_n > 1 else contextlib.nullcontext()
            )
            loop_ctx.__enter__()
            if not dma_outside:
                dma_inputs()

            # ---------------- Phase A: QKV projection -----------------
            def emit_qkv_qk(hp):
                for t in range(2):  # q, k -> [channels, n] layout
                    j = t * 4 + hp
                    for nh in range(2):
                        ps = qkv_psum()
                        for cc in range(8):
                            nc.tensor.matmul(
                                ps[:],
                                wqkv[:, cc, j * 128 : (j + 1) * 128],
                                xT[:, cc, nh * 512 : (nh + 1) * 512],
                                start=(cc == 0),
                                stop=(cc == 7),
                            )
                        nc.scalar.copy(qkvT[:, j, nh * 512 : (nh + 1) * 512], ps[:])

            def emit_v():
                # v -> [keys, channels] layout (operands swapped)
                for mc in range(8):
                    ps = qkv_psum()
                    for cc in range(8):
                        nc.tensor.matmul(
                            ps[:],
                            xT[:, cc, mc * 128 : (mc + 1) * 128],
                            wqkv[:, cc, 1024:1536],
                            start=(cc == 0),
                            stop=(cc == 7),
                        )
                    nc.scalar.copy(
                        vsb[:, mc, :, 0:64], ps.rearrange("p (g d) -> p g d", g=8)
                    )

            if "qkv" not in skip:
                emit_v()
                emit_qkv_qk(0)

            # ---------------- Phase B: attention ----------------------
            # combo order ci: 0=(even,h1) 1=(odd,h1) 2=(even,h2) 3=(odd,h2)
            # (rg, parity): rg = score row-group, parity selects V head
            combo = [(0, 0), (2, 1), (1, 0), (3, 1)]
            if True:
                for hp in range(4) if "attn" not in skip else []:
                    for nh in range(2):
                        pv = pV.tile([65, 4, 512], f32, tag="pv")

                        def emit_pv(mc, etiles):
                            for g in range(2):
                                for i in range(2):
                                    ci = 2 * g + i
                                    _rg, par = combo[ci]
                                    nc.tensor.matmul(
                                        pv[:, ci, :],
                                        vsb[:, mc, 2 * hp + par, :],
                                        etiles[g][:, i, :],
                                        start=(mc == 0),
                                        stop=(mc == 7),
                                    )

                        # software pipeline: PV for chunk mc is emitted after
                        # the scores/exp of chunk mc+1, so the PE never stalls
                        # on the current chunk's exp
                        prev = None
                        for mc in range(8):
                            cur = []
                            for g in range(2):
                                s_ps = pS.tile([128, 2, 512], f32, tag="s")
                                for i in range(2):
                                    rg, _par = combo[2 * g + i]
                                    nc.tensor.matmul(
                                        s_ps[:, i, :],
                                        qkvT[
                                            32 * rg : 32 * rg + 32,
                                            4 + hp,
                                            mc * 128 : (mc + 1) * 128,
                                        ],
                                        qkvT[
                                            32 * rg : 32 * rg + 32,
                                            hp,
                                            nh * 512 : (nh + 1) * 512,
                                        ],
                                        start=True,
                                        stop=True,
                                        tile_position=(32 * rg, 0),
                                    )
                                e_sb = epool.tile([128, 2, 512], b16, tag="e")
                                nc.scalar.activation(
                                    e_sb[:], s_ps[:], Exp, scale=0.125
                                )
                                cur.append(e_sb)
                            if prev is not None:
                                emit_pv(mc - 1, prev)
                            prev = cur
                        emit_pv(7, prev)
                        # evacuate pv to SBUF immediately so the next sweep's
                        # PV accumulation can reuse the PSUM banks; the whole
                        # combine below then runs off the critical path
                        pvs = wpool.tile([65, 4, 512], f32, tag="pvs")
                        nc.vector.tensor_copy(pvs[:], pv[:])
                        # combine: O = O1/d1 - lam*O2/d2
                        rsb = wpool.tile([1, 4, 512], b16, tag="rsb")
                        with nc.allow_low_precision(
                            reason="bf16 softmax denominator reciprocals"
                        ):
                            nc.vector.reciprocal(rsb[:], pvs[64:65, :, :])
                        nc.vector.tensor_scalar_mul(
                            rsb[0:1, 2:4, :], rsb[0:1, 2:4, :], neglam[:]
                        )
                        # broadcast recips across partitions via PE ones-matmul
                        # (reuses freed score-pool slots), then stage in SBUF
                        rbc = wpool.tile([64, 4, 512], f32, tag="rbc")
                        for g in range(2):
                            rbc_ps = pS.tile([128, 2, 512], f32, tag="s")
                            for i in range(2):
                                nc.tensor.matmul(
                                    rbc_ps[0:64, i, :],
                                    ones1[:],
                                    rsb[0:1, 2 * g + i, :],
                                    start=True,
                                    stop=True,
                                )
                            nc.vector.tensor_copy(
                                rbc[:, 2 * g : 2 * g + 2, :], rbc_ps[0:64, :, :]
                            )
                        for par in range(2):
                            c1, c2 = par, 2 + par
                            t0 = wpool.tile([64, 512], f32, tag="t0")
                            t1 = wpool.tile([64, 512], f32, tag="t1")
                            nc.vector.tensor_mul(
                                out=t0[:], in0=pvs[0:64, c1, :], in1=rbc[:, c1, :]
                            )
                            nc.vector.tensor_mul(
                                out=t1[:], in0=pvs[0:64, c2, :], in1=rbc[:, c2, :]
                            )
                            nc.vector.tensor_add(
                                out=oT[
                                    par * 64 : (par + 1) * 64,
                                    hp,
                                    nh * 512 : (nh + 1) * 512,
                                ],
                                in0=t0[:],
                                in1=t1[:],
                            )
                        if nh == 0 and hp < 3 and "qkv" not in skip:
                            # interleave next head-pair's q/k projection into
                            # this head-pair's attention stream (fills PE gaps)
                            emit_qkv_qk(hp + 1)

            # ---------------- Phase C: output projection --------------
            if True:
                for ncc in range(8) if "proj" not in skip else []:
                    for jh in range(2):
                        ps = qkv_psum()
                        for ci in range(4):
                            nc.tensor.matmul(
                                ps[:],
                                oT[:, ci, ncc * 128 : (ncc + 1) * 128],
                                wp[:, ci, jh * 512 : (jh + 1) * 512],
                                start=(ci == 0),
                                stop=(ci == 3),
                            )
                        osb = wpool.tile([128, 512], f32, tag="osb")
                        nc.scalar.copy(osb[:], ps[:])
                        nc.sync.dma_start(
                            out_d[ncc, :, jh * 512 : (jh + 1) * 512], osb[:]
                        )

            loop_ctx.__exit__(None, None, None)

    nc.compile()
    _PROG_CACHE[key] = nc
    return nc


def _prep_core_inputs(x, W_qkv, W_proj, neg_lam):
    """Host-side shard prep. Returns in_maps for the 8 cores."""
    W4 = np.asarray(W_qkv, np.float32).reshape(3, H, HD, C)
    in_maps = []
    for core in range(8):
        b, hg = divmod(core, 2)
        xT = (
            np.ascontiguousarray(np.asarray(x[b], np.float32).T)
            .reshape(8, 128, N)
            .astype(BF16)
        )
        wsl = W4[:, hg * 8 : (hg + 1) * 8]  # [3, 8, 64, 1024]
        wqkv = (
            np.ascontiguousarray(wsl.transpose(3, 0, 1, 2).reshape(C, 1536))
            .reshape(8, 128, 1536)
            .astype(BF16)
        )
        wp = (
            np.ascontiguousarray(
                np.asarray(W_proj, np.float32)[:, hg * 512 : (hg + 1) * 512].T
            )
            .reshape(4, 128, C)
            .astype(BF16)
        )
        in_maps.append(
            {
                "xT": xT,
                "wqkv": wqkv,
                "wp": wp,
                "neglam": np.full((1, 1), neg_lam, np.float32),
            }
        )
    return in_maps


def kernel(x, W_qkv, W_proj, b_proj, lambda_q1, lambda_k1, lambda_q2, lambda_k2):
    from concourse.bass_utils import run_bass_kernel_spmd

    lq1 = np.asarray(lambda_q1, np.float64)
    lk1 = np.asarray(lambda_k1, np.float64)
    lq2 = np.asarray(lambda_q2, np.float64)
    lk2 = np.asarray(lambda_k2, np.float64)
    lam = float(np.mean(np.exp(lq1 * lk1) - np.exp(lq2 * lk2) + LAMBDA_INIT))

    nc = _build_program()
    in_maps = _prep_core_inputs(x, W_qkv, W_proj, -lam)
    res = run_bass_kernel_spmd(nc, in_maps, core_ids=list(range(8)))
    _PROG_CACHE["last_result"] = res

    bp = np.asarray(b_proj, np.float32)
    out = np.empty((B, N, C), np.float32)
    for b in range(B):
        p0 = res.results[2 * b]["out"].reshape(N, C)
        p1 = res.results[2 * b + 1]["out"].reshape(N, C)
        out[b] = p0 + p1 + bp[None, :]
    return out


# revision 55
# speedup vs baseline: 1.0967x; 1.0960x over previous
"""DifferentialAttention Trainium2 kernel (8 NeuronCores, SPMD).

Sharding: data-parallel over batch B=4, tensor-parallel over heads
(2 cores per batch element, 8 heads each).  Each core computes the
partial projection output for its 8 heads; the host sums the two
partials per batch element and adds b_proj.

Per-core device pipeline (all matmuls bf16 inputs, fp32 PSUM accum):
  1. QKV^T = W_slice^T.T @ x^T            -> [channels, n] layout
  2. V transpose via PE (keys on partitions), ones column appended
  3. scores S^T[m, n] per (head, half) with 4-way row-group packing
     (contraction d=32 -> PE row groups 0/32/64/96)
  4. exp on ScalarE (scale=1/8 folded in), bf16 out
  5. PV:  [V | 1]^T @ E  -> unnormalized out^T + softmax denominator row
  6. combine: O^T = O1/d1 - lam*O2/d2 (reciprocal + GPSIMD partition
     broadcast + DVE mul/add)
  7. proj: out = O^T.T @ Wp_slice
"""

import sys

sys.path.insert(0, "/opt/trn_rl_repo")

import numpy as np
import ml_dtypes

B, N, C, H, HD = 4, 1024, 1024, 16, 64
LAMBDA_INIT = 0.8
BF16 = ml_dtypes.bfloat16

_PROG_CACHE = {}


def _build_program(loop_n=1, dma_outside=False, skip=()):
    key = ("nc", loop_n, dma_outside, tuple(skip))
    if key in _PROG_CACHE:
        return _PROG_CACHE[key]

    import contextlib

    import concourse.mybir as mybir
    import concourse.tile as tile
    from concourse import bacc

    f32 = mybir.dt.float32
    b16 = mybir.dt.bfloat16
    Exp = mybir.ActivationFunctionType.Exp

    nc = bacc.Bacc(None)

    x_d = nc.dram_tensor("xT", [8, 128, N], b16, kind="ExternalInput")
    wqkv_d = nc.dram_tensor("wqkv", [8, 128, 1536], b16, kind="ExternalInput")
    wp_d = nc.dram_tensor("wp", [4, 128, C], b16, kind="ExternalInput")
    neglam_d = nc.dram_tensor("neglam", [1, 1], f32, kind="ExternalInput")
    out_d = nc.dram_tensor("out", [8, 128, C], f32, kind="ExternalOutput")

    with tile.TileContext(nc) as tc:
        with (
            tc.tile_pool(name="io", bufs=1) as iopool,
            tc.tile_pool(name="work", bufs=4) as wpool,
            tc.tile_pool(name="esb", bufs=6) as epool,
            tc.tile_pool(name="pS", bufs=2, space="PSUM") as pS,
            tc.tile_pool(name="pV", bufs=1, space="PSUM") as pV,
        ):
            xT = iopool.tile([128, 8, N], b16)
            wqkv = iopool.tile([128, 8, 1536], b16)
            wp = iopool.tile([128, 4, C], b16)
            neglam = iopool.tile([1, 1], f32)
            ones1 = iopool.tile([1, 64], b16)
            # qkvT chunk j: j in 0..3 -> q head-pair j; 4..7 -> k head-pair
            # within a chunk: partitions 0-63 even head (d 0..63), 64-127 odd
            qkvT = iopool.tile([128, 8, N], b16)
            # V in [keys, channels] layout: [m%128, m//128, head_local, d|1]
            # col 64 of each head's block is the ones column (softmax denom)
            vsb = iopool.tile([128, 8, 8, 65], b16)
            # combined attention output (transposed): [cl, hp, n]
            oT = iopool.tile([128, 4, N], b16)

            nc.gpsimd.memset(ones1[:], 1.0)
            nc.gpsimd.memset(vsb[:, :, :, 64:65], 1.0)
            if "attn" in skip:
                nc.gpsimd.memset(oT[:], 0.0)

            def dma_inputs():
                # split across both HWDGE rings (SP + ACT) for 2x issue width
                for cc in range(8):
                    nc.sync.dma_start(xT[:, cc, :], x_d[cc])
                    nc.scalar.dma_start(wqkv[:, cc, :], wqkv_d[cc])
                for ci in range(4):
                    nc.sync.dma_start(wp[:, ci, :], wp_d[ci])
                nc.sync.dma_start(neglam[:], neglam_d[:])

            def qkv_psum():
                # phases A/C borrow score-pool slots ([128,2,512] granularity)
                t = pS.tile([128, 2, 512], f32, tag="s", name="qkvps")
                return t[:, 0, :]

            if dma_outside:
                dma_inputs()
            loop_ctx = (
                tc.For_i(0, loop_n, 1) if loop_n > 1 else contextlib.nullcontext()
            )
            loop_ctx.__enter__()
            if not dma_outside:
                dma_inputs()

            # ---------------- Phase A: QKV projection -----------------
            def emit_qkv_qk(hp, on_act=True):
                # evacuation engine: ScalarE when ACT is idle (phase A),
                # DVE when interleaved into the attention stream
                evac = nc.scalar.copy if on_act else nc.vector.tensor_copy
                for t in range(2):  # q, k -> [channels, n] layout
                    j = t * 4 + hp
                    for nh in range(2):
                        ps = qkv_psum()
                        for cc in range(8):
                            nc.tensor.matmul(
                                ps[:],
                                wqkv[:, cc, j * 128 : (j + 1) * 128],
                                xT[:, cc, nh * 512 : (nh + 1) * 512],
                                start=(cc == 0),
                                stop=(cc == 7),
                            )
                        evac(qkvT[:, j, nh * 512 : (nh + 1) * 512], ps[:])

            def emit_v():
                # v -> [keys, channels] layout (operands swapped)
                for mc in range(8):
                    ps = qkv_psum()
                    for cc in range(8):
                        nc.tensor.matmul(
                            ps[:],
                            xT[:, cc, mc * 128 : (mc + 1) * 128],
                            wqkv[:, cc, 1024:1536],
                            start=(cc == 0),
                            stop=(cc == 7),
                        )
                    nc.scalar.copy(
                        vsb[:, mc, :, 0:64], ps.rearrange("p (g d) -> p g d", g=8)
                    )

            if "qkv" not in skip:
                emit_v()
                emit_qkv_qk(0)

            # ---------------- Phase B: attention ----------------------
            # combo order ci: 0=(even,h1) 1=(odd,h1) 2=(even,h2) 3=(odd,h2)
            # (rg, parity): rg = score row-group, parity selects V head
            combo = [(0, 0), (2, 1), (1, 0), (3, 1)]
            if True:
                for hp in range(4) if "attn" not in skip else []:
                    for nh in range(2):
                        pv = pV.tile([65, 4, 512], f32, tag="pv")

                        def emit_pv(mc, etiles):
                            for g in range(2):
                                for i in range(2):
                                    ci = 2 * g + i
                                    _rg, par = combo[ci]
                                    nc.tensor.matmul(
                                        pv[:, ci, :],
                                        vsb[:, mc, 2 * hp + par, :],
                                        etiles[g][:, i, :],
                                        start=(mc == 0),
                                        stop=(mc == 7),
                                    )

                        # software pipeline: PV for chunk mc is emitted after
                        # the scores/exp of chunk mc+1, so the PE never stalls
                        # on the current chunk's exp
                        prev = None
                        for mc in range(8):
                            cur = []
                            for g in range(2):
                                s_ps = pS.tile([128, 2, 512], f32, tag="s")
                                for i in range(2):
                                    rg, _par = combo[2 * g + i]
                                    nc.tensor.matmul(
                                        s_ps[:, i, :],
                                        qkvT[
                                            32 * rg : 32 * rg + 32,
                                            4 + hp,
                                            mc * 128 : (mc + 1) * 128,
                                        ],
                                        qkvT[
                                            32 * rg : 32 * rg + 32,
                                            hp,
                                            nh * 512 : (nh + 1) * 512,
                                        ],
                                        start=True,
                                        stop=True,
                                        tile_position=(32 * rg, 0),
                                    )
                                e_sb = epool.tile([128, 2, 512], b16, tag="e")
                                nc.scalar.activation(
                                    e_sb[:], s_ps[:], Exp, scale=0.125
                                )
                                cur.append(e_sb)
                            if prev is not None:
                                emit_pv(mc - 1, prev)
                            prev = cur
                        emit_pv(7, prev)
                        # combine: O = O1/d1 - lam*O2/d2.  Reciprocal reads the
                        # denominator row straight from PSUM so it doesn't wait
                        # on the pv evacuation; pv is evacuated in parallel so
                        # the next sweep can reuse the PV banks.
                        rsb = wpool.tile([1, 4, 512], b16, tag="rsb")
                        with nc.allow_low_precision(
                            reason="bf16 softmax denominator reciprocals"
                        ):
                            nc.vector.reciprocal(rsb[:], pv[64:65, :, :])
                        nc.gpsimd.tensor_scalar_mul(
                            rsb[0:1, 2:4, :], rsb[0:1, 2:4, :], neglam[:]
                        )
                        # broadcast recips across partitions via PE ones-
                        # matmuls, col-packed two-per-bank into a single
                        # borrowed score-pool slot (keeps the other slot free
                        # for the next sweep's scores)
                        rbc_ps = pS.tile([128, 2, 512], f32, tag="s")
                        for ci in range(4):
                            base = 64 * (ci % 2)
                            nc.tensor.matmul(
                                rbc_ps[base : base + 64, ci // 2, :],
                                ones1[:],
                                rsb[0:1, ci, :],
                                start=True,
                                stop=True,
                                tile_position=(0, base),
                            )
                        # stage to two base-0 SBUF tiles (GPSIMD TT requires
                        # matching base partitions for both SBUF inputs)
                        rbc_a = wpool.tile([64, 2, 512], f32, tag="rbc_a")
                        rbc_b = wpool.tile([64, 2, 512], f32, tag="rbc_b")
                        nc.vector.tensor_copy(rbc_a[:], rbc_ps[0:64, :, :])
                        nc.vector.tensor_copy(rbc_b[:], rbc_ps[64:128, :, :])
                        pvs = wpool.tile([65, 4, 512], f32, tag="pvs")
                        nc.vector.tensor_copy(pvs[:], pv[:])
                        for par in range(2):
                            # combo ci -> (rbc_b if ci odd else rbc_a)[ci//2]
                            c1, c2 = par, 2 + par
                            r1 = (rbc_b if c1 % 2 else rbc_a)[:, c1 // 2, :]
                            r2 = (rbc_b if c2 % 2 else rbc_a)[:, c2 // 2, :]
                            t0 = wpool.tile([64, 512], f32, tag="t0")
                            t1 = wpool.tile([64, 512], f32, tag="t1")
                            nc.gpsimd.tensor_mul(
                                out=t0[:], in0=pvs[0:64, c1, :], in1=r1
                            )
                            nc.gpsimd.tensor_mul(
                                out=t1[:], in0=pvs[0:64, c2, :], in1=r2
                            )
                            nc.gpsimd.tensor_add(
                                out=oT[
                                    par * 64 : (par + 1) * 64,
                                    hp,
                                    nh * 512 : (nh + 1) * 512,
                                ],
                                in0=t0[:],
                                in1=t1[:],
                            )
                        if nh == 0 and hp < 3 and "qkv" not in skip:
                            # interleave next head-pair's q/k projection into
                            # this head-pair's attention stream (fills PE gaps)
                            emit_qkv_qk(hp + 1, on_act=False)

            # ---------------- Phase C: output projection --------------
            if True:
                for ncc in range(8) if "proj" not in skip else []:
                    for jh in range(2):
                        ps = qkv_psum()
                        for ci in range(4):
                            nc.tensor.matmul(
                                ps[:],
                                oT[:, ci, ncc * 128 : (ncc + 1) * 128],
                                wp[:, ci, jh * 512 : (jh + 1) * 512],
                                start=(ci == 0),
                                stop=(ci == 3),
                            )
                        osb = wpool.tile([128, 512], f32, tag="osb")
                        nc.scalar.copy(osb[:], ps[:])
                        nc.sync.dma_start(
                            out_d[ncc, :, jh * 512 : (jh + 1) * 512], osb[:]
                        )

            loop_ctx.__exit__(None, None, None)

    nc.compile()
    _PROG_CACHE[key] = nc
    return nc


def _prep_core_inputs(x, W_qkv, W_proj, neg_lam):
    """Host-side shard prep. Returns in_maps for the 8 cores."""
    W4 = np.asarray(W_qkv, np.float32).reshape(3, H, HD, C)
    in_maps = []
    for core in range(8):
        b, hg = divmod(core, 2)
        xT = (
            np.ascontiguousarray(np.asarray(x[b], np.float32).T)
            .reshape(8, 128, N)
            .astype(BF16)
        )
        wsl = W4[:, hg * 8 : (hg + 1) * 8]  # [3, 8, 64, 1024]
        wqkv = (
            np.ascontiguousarray(wsl.transpose(3, 0, 1, 2).reshape(C, 1536))
            .reshape(8, 128, 1536)
            .astype(BF16)
        )
        wp = (
            np.ascontiguousarray(
                np.asarray(W_proj, np.float32)[:, hg * 512 : (hg + 1) * 512].T
            )
            .reshape(4, 128, C)
            .astype(BF16)
        )
        in_maps.append(
            {
                "xT": xT,
                "wqkv": wqkv,
                "wp": wp,
                "neglam": np.full((1, 1), neg_lam, np.float32),
            }
        )
    return in_maps


def kernel(x, W_qkv, W_proj, b_proj, lambda_q1, lambda_k1, lambda_q2, lambda_k2):
    from concourse.bass_utils import run_bass_kernel_spmd

    lq1 = np.asarray(lambda_q1, np.float64)
    lk1 = np.asarray(lambda_k1, np.float64)
    lq2 = np.asarray(lambda_q2, np.float64)
    lk2 = np.asarray(lambda_k2, np.float64)
    lam = float(np.mean(np.exp(lq1 * lk1) - np.exp(lq2 * lk2) + LAMBDA_INIT))

    nc = _build_program()
    in_maps = _prep_core_inputs(x, W_qkv, W_proj, -lam)
    res = run_bass_kernel_spmd(nc, in_maps, core_ids=list(range(8)))
    _PROG_CACHE["last_result"] = res

    bp = np.asarray(b_proj, np.float32)
    out = np.empty((B, N, C), np.float32)
    for b in range(B):
        p0 = res.results[2 * b]["out"].reshape(N, C)
        p1 = res.results[2 * b + 1]["out"].reshape(N, C)
        out[b] = p0 + p1 + bp[None, :]
    return out
